# revision 31
# baseline (speedup 1.0000x reference)
"""Trainium2 Bass kernel for nn_NSMCell (GNN message passing).

Strategy
--------
The reference output is only [N]: a per-graph blend of two segment softmaxes
over per-node scalars.  Both scalars are of the form

    s_i = sum_d w_d * elu( M_g[d, :] @ x_i )

where for "node items" M_g = (sim[g] . W_node_props) * instr[g] and x = node
attr, and for "edge items" M_g = W_edge * instr[g] and x = edge attr.  The
per-graph matrices are built on the host (they are tiny); the device streams
all item columns through matmuls + an elu drain + a weighted partition
reduce.  The edge-message scatter (index_add) collapses to a host-side
bincount of per-edge scalars, and the segment softmax + blend run on the
host over [N] values (negligible work).

Sharding: graphs are ranked by edge count and dealt round-robin so core d
gets slot-j graph rank 8j+d.  All 8 cores share one NEFF: per-slot run
lengths are the max over the 8 cores' graphs in that slot.  Items are
packed [edge slots 8-15 | all node runs | edge slots 0-7] so only the
first 1 MB weight chunk gates startup.

Device layout per 512-item tile (d on partitions, 2 chunks of 128 side by
side in one 2-bank PSUM tile):
  y[d, e]   = A_seg[k, d]^T @ xT[k, e]      2 matmuls per (piece, dc) -> PSUM

The elu(y)+1 drain is split across engines so neither ACT nor DVE is a
wall (any PSUM-reading pass runs at ~1 col/cycle with no 2x mode):
  - "poly" tiles (first N_A): one custom DVE op computes
        EL1 = min(relu(c1*y + c2)^4, relu(y) + 1)  ~= elu(y)+1
    in a single pass (max abs err ~0.03, distribution rms ~0.01; errors
    are averaged over 256 d's by the w-reduce so the softmax impact is
    ~0.3%).  Tiles are contiguous so most graphs see a homogeneous path.
  - "exact" tiles (rest): ACT Relu pass -> T1, ACT Exp pass -> E, then a
    fused scalar_tensor_tensor  EL1 = min(E, 1) + T1  on GPSIMD/DVE
    (bf16 SBUF, cheap).  Identity: elu(y)+1 = relu(y) + min(e^y, 1).

The s-reduce  s_row += (w (x) delta_c)^T @ EL1  uses M=32 matmuls which
waste 3/4 of the PE array; the 128x128 array is 16 independent 32x32
subarrays, so s-matmuls for 4 consecutive tiles are issued back-to-back
into 4 distinct 32-column groups (tile_position) and run concurrently:
tile t -> col group t%4, row (t//4)%32 selected by the (w (x) delta_c)
weight.  The s-bank is drained once at the end; the host subtracts
sum(w) to undo the +1.

Item DMAs alternate between the SP and Pool HWDGE queues plus the ACT
queue; one queue caps at ~175 GB/s, two sustain ~260 GB/s.
"""

import numpy as np
import ml_dtypes

BF16 = ml_dtypes.bfloat16
N_CORES = 8
D = 256
TILE = 512  # items per tile

# elu(y)+1 ~= min(relu(PC1*y + PC2)^4, relu(y)+1)  (minimax fit: unbiased
# enough that graphs mixing poly and exact tiles stay accurate)
PC1 = 0.20922107052487887
PC2 = 0.9984114123856602
# tile classes: "a" = single-pass DVE poly drain, "b" = exact ACT relu/exp
# pair.  Interleave so DVE and ACT run concurrently (DVE/ACT balance at
# a:b ~ 2100:1141 ~ 11:6), never adjacent (a b-tile holds its y PSUM for
# ~2.2us of ACT chain; clustering them stalls the 3-deep y pipeline).
_B_POS = {1, 4, 7, 10, 13, 15}  # within a period of 17
A_TAIL = 8  # last tiles forced to the short poly chain to shrink the tail


def _is_b_tile(t, n_tiles=1 << 30):
    return (t % 17) in _B_POS and t < n_tiles - A_TAIL


FLUSH_AT = 8      # s-matmul batch-of-4 issue slack (in tiles)


# ----------------------------------------------------------------------------
# Bass kernel builder (one NEFF shared by all cores)
# ----------------------------------------------------------------------------

_BASS_CACHE = {}


def _get_polyel1_op():
    """Register (once) a custom fused DVE op:

        out = min(relu(in0*s0 + s1)^4, relu(in0) + 1)

    With in0 = y this computes elu(y)+1 to ~0.03 abs err in one VectorE
    pass straight out of PSUM, replacing the ACT exp + DVE min pair."""
    from concourse import dve_ops
    from concourse.dve_spec import (Spec, Src0, C0, C1, One, relu, sq, minn,
                                    lower, _has_src1)
    from concourse.dve_uop import DveOpSpec

    for o in dve_ops.OPS:
        if o.name == "POLYEL1_ANT":
            return o

    def ref(in0, in1, s0, s1, imm2):
        x = in0.astype(np.float32)
        g = np.maximum(x * s0 + s1, 0.0) ** 4
        return np.minimum(g, np.maximum(x, 0.0) + 1.0).astype(np.float32)

    body = minn(sq(sq(relu(Src0 * C0 + C1))), relu(Src0) + One)
    spec = Spec(body=body, reference=ref)
    row = dve_ops._CUSTOM_DVE_ROW_BASE + len(dve_ops.OPS)
    shas = {}
    for ver in ("v3", "v4"):
        uops = lower(spec, ver=ver)
        shas[ver] = DveOpSpec(name="POLYEL1_ANT", opcode=row, uops=uops,
                              rd1_en=_has_src1(spec)).sha(ver)
    op = dve_ops.DveOp("POLYEL1_ANT", spec, subdim=False, uops_sha=shas)
    dve_ops.OPS.append(op)
    dve_ops.CUSTOM_DVE_SPECS[op.name] = op.spec
    dve_ops._SUB_OPCODE_FOR_NAME[op.name] = row
    return op


def _build_bass(n_tiles, pieces, stypes):
    """Build the Tile/Bass program.

    n_tiles: number of 512-item tiles per core
    pieces:  per tile, tuple of (a, b, u): y-matmul column range [a, b) using
             per-graph matrix u (u = slot for nodes, 16 + slot for edges)
    stypes:  per tile, tuple of (ra, rb, typ): s-reduce column ranges by
             item type (0 = node -> w_node, 1 = edge -> w_rel)
    """
    key = (n_tiles, pieces, stypes, tuple(sorted(_B_POS)), A_TAIL)
    if key in _BASS_CACHE:
        return _BASS_CACHE[key]

    import concourse.mybir as mybir
    import concourse.tile as tile
    from concourse import bacc

    dt = mybir.dt
    n_seg = 32  # 16 slots x (node, edge), u-ordered: nodes 0-15, edges 16-31
    assert n_tiles <= 128

    poly = _get_polyel1_op()
    nc = bacc.Bacc("TRN2", target_bir_lowering=False)
    m_pad = n_tiles * TILE
    items_d = nc.dram_tensor("items", [128, 2 * m_pad], dt.bfloat16,
                             kind="ExternalInput")
    mats_d = nc.dram_tensor("mats", [128, n_seg * 2 * 2 * 128], dt.bfloat16,
                            kind="ExternalInput")
    wtab_d = nc.dram_tensor("wtab", [128, 2 * 2 * 32 * 32], dt.bfloat16,
                            kind="ExternalInput")
    s_d = nc.dram_tensor("s_out", [128, TILE], dt.float32,
                         kind="ExternalOutput")

    with tile.TileContext(nc) as tc:
        with (
            tc.tile_pool(name="const", bufs=1) as const_pool,
            tc.tile_pool(name="items", bufs=16) as item_pool,
            tc.tile_pool(name="psum_y", bufs=3, space="PSUM") as ypool,
            tc.tile_pool(name="psum_s", bufs=1, space="PSUM") as spool,
            tc.tile_pool(name="elu", bufs=10) as elu_pool,
            tc.tile_pool(name="bt", bufs=6) as bt_pool,
            tc.tile_pool(name="sout", bufs=1) as sout_pool,
            tc.tile_pool(name="warm", bufs=1, space="PSUM") as warm_pool,
        ):
            # Consts: one pool tile per chunk so the chunk DMAs carry no
            # same-tile WAW deps (they'd serialize otherwise).
            MCH = 8  # u-slots per mats chunk
            mats_sbs = [const_pool.tile([128, MCH * 512], dt.bfloat16,
                                        name=f"matsb{i}", tag=f"mats{i}")
                        for i in range(n_seg // MCH)]
            wtab_sbs = [const_pool.tile([128, 8 * 128], dt.bfloat16,
                                        name=f"wtabb{i}", tag=f"wtab{i}")
                        for i in range(4)]

            def load_mats(ch, eng=None):
                sl = slice(ch * MCH * 512, (ch + 1) * MCH * 512)
                (eng or nc.scalar).dma_start(mats_sbs[ch][:], mats_d[:, sl])

            def load_mats_q(ch, q, eng):
                # one quarter of a mats chunk - slipped between item DMAs
                w = MCH * 512 // 4
                sl = slice(ch * MCH * 512 + q * w, ch * MCH * 512 + (q + 1) * w)
                eng.dma_start(mats_sbs[ch][:, q * w:(q + 1) * w],
                              mats_d[:, sl])

            def load_wtab(ch):
                sl = slice(ch * 8 * 128, (ch + 1) * 8 * 128)
                nc.scalar.dma_start(wtab_sbs[ch][:, :], wtab_d[:, sl])

            # HAM pre-warm: keep PE busy during the DMA preamble so real
            # matmuls start at 2.4 GHz instead of ramping from 1.2 GHz.
            # gpsimd issues its first instruction ~1us before the other
            # engines, so the warm block starts earliest off its memset;
            # ~75 warm matmuls bridge until the first item lands so the HAM
            # un-throttles right as real matmuls begin.
            warm_sb = const_pool.tile([128, 64], dt.bfloat16)
            nc.gpsimd.memset(warm_sb[:], 0)
            warm_ps = warm_pool.tile([128, 64], dt.float32)
            for _ in range(75):
                nc.tensor.matmul(warm_ps[0:64, :], warm_sb[:], warm_sb[:],
                                 start=True, stop=True, skip_group_check=True)

            # Startup: items stream on the SP+Pool queues from tile 0 while
            # the ACT queue carries mats chunk 0 (quartered so the first
            # y-matmuls release as soon as their slice lands) and wtab 0.
            # Everything else is deferred near its first use so the three
            # queues are item-dedicated once the pipeline is hot.
            first_use = {ch: n_tiles for ch in range(4)}
            for t in range(n_tiles):
                for (_, _, u) in pieces[t]:
                    first_use[u // MCH] = min(first_use[u // MCH], t)
            const_sched = {}

            def sched(tq, fn):
                const_sched.setdefault(tq, []).append(fn)

            # mats ch0 in eighths: the first eighth (the weights of tile 0)
            # leads the SP queue ahead of its first item; the rest stream on
            # the ACT queue which carries no items until tile 11.
            def load_mats_8(ch, q, eng):
                w = MCH * 512 // 8
                sl = slice(ch * MCH * 512 + q * w, ch * MCH * 512 + (q + 1) * w)
                eng.dma_start(mats_sbs[ch][:, q * w:(q + 1) * w], mats_d[:, sl])

            sched(0, lambda: load_mats_8(0, 0, nc.sync))
            for q in range(1, 8):
                sched(q - 1, lambda q=q: load_mats_8(0, q, nc.scalar))
            sched(7, lambda: load_wtab(0))

            # all on the ACT queue: it only carries 1 in 3 item tiles after
            # joining the rotation, so const quarters slot into its gaps
            # without stealing item bandwidth from the SP/Pool queues
            for ch, t0 in ((1, 8), (2, 16), (3, 24)):
                t0 = min(t0, max(2, first_use[ch] - 12))
                for q in range(4):
                    sched(t0 + 2 * q, lambda ch=ch, q=q:
                          load_mats_q(ch, q, nc.scalar))
            # wtab chunk g is first used at tile 32g (s-row c = (t//4)%32)
            sched(20, lambda: load_wtab(1))
            if n_tiles > 64:
                sched(48, lambda: load_wtab(2))
            if n_tiles > 96:
                sched(72, lambda: load_wtab(3))
            psum_s = spool.tile([128, TILE], dt.float32)

            def mat_sl(u, kc, dc):
                ch, s = divmod(u, MCH)
                off = ((s * 2 + kc) * 2 + dc) * 128
                return mats_sbs[ch][:, off:off + 128]

            def w_sl(typ, kc, c):
                ch, cc = divmod(c, 8)
                off = ((cc * 2 + typ) * 2 + kc) * 32
                return wtab_sbs[ch][:, off:off + 32]

            # s-matmul col-group schedule: tile t -> col group g = t%4 (PSUM
            # partitions 32g..32g+32), row within group c = (t//4)%32 picked
            # by the (w (x) delta_c) weight.  Batches of 4 consecutive tiles
            # issue back-to-back so the 4 col groups compute concurrently.
            strip_started = set()
            strip_last = {g: max(range(g, n_tiles, 4)) for g in range(4)
                          if g < n_tiles}
            pending = []  # (t, [el parts to accumulate])
            bt_mins = {}  # t -> deferred min(e,1) emitters

            def flush_batch():
                batch = pending[:4]
                del pending[:4]
                # deferred b-tile mins: they only feed this batch's s-matmuls
                # so they must not sit in DVE's FIFO ahead of the PSUM-
                # critical poly drains
                for (t, parts) in batch:
                    for fn in bt_mins.pop(t, ()):
                        fn()
                for kc in range(2):
                    for (t, parts) in batch:
                        g, c = t % 4, (t // 4) % 32
                        out_rows = psum_s[32 * g:32 * g + 32, :]
                        tp = (0, 32 * g)
                        srs = stypes[t]
                        for pi, el_t in enumerate(parts):
                            for ri, (ra, rb, typ) in enumerate(srs):
                                st = g not in strip_started
                                strip_started.add(g)
                                stop = (t == strip_last[g] and kc == 1
                                        and pi == len(parts) - 1
                                        and ri == len(srs) - 1)
                                nc.tensor.matmul(
                                    out_rows[:, ra:rb], w_sl(typ, kc, c),
                                    el_t[:, kc * TILE + ra:kc * TILE + rb],
                                    start=st, stop=stop,
                                    tile_position=tp, skip_group_check=True)

            for t in range(n_tiles):
                for fn in const_sched.get(t, ()):
                    fn()
                x2 = item_pool.tile([128, 2 * TILE], dt.bfloat16, tag="x")
                xoff = 0
                # early: Pool gets the even tiles (its queue is otherwise
                # empty; SP's first item rides behind mats q0)
                if t < 9:
                    eng = nc.gpsimd if t % 2 == 0 else nc.sync
                else:
                    eng = (nc.sync, nc.gpsimd, nc.scalar)[t % 3]
                eng.dma_start(
                    x2[:], items_d[:, t * 2 * TILE:(t + 1) * 2 * TILE])

                # both d-chunks side by side in one 2-bank PSUM tile.
                # Snake the (dc, kc) order across tiles: within a graph run
                # the weights repeat tile-to-tile, so the first matmul of
                # tile t reuses the weight the last matmul of t-1 loaded and
                # doesn't wait on a fresh LDWEIGHTS.
                y = ypool.tile([128, 2 * TILE], dt.float32, tag="y")
                n_p = len(pieces[t])
                combos = [(0, 0), (0, 1), (1, 1), (1, 0)]  # (dc, kc)
                if t % 2:
                    combos = combos[::-1]
                for pi, (a, b, u) in enumerate(pieces[t]):
                    for ci, (dc, kc) in enumerate(combos):
                        # ci 0/2 are each dc-half's first write: start=True
                        # marks that half's PSUM bank pending-zero
                        ysl = y[:, dc * TILE + a:dc * TILE + b]
                        nc.tensor.matmul(
                            ysl, mat_sl(u, kc, dc),
                            x2[:, xoff + kc * TILE + a:xoff + kc * TILE + b],
                            start=(pi == 0 and ci in (0, 2)),
                            stop=(pi == n_p - 1 and ci in (1, 3)),
                            skip_group_check=True)

                if not _is_b_tile(t, n_tiles):
                    # single-pass poly drain on DVE
                    el_t = elu_pool.tile([128, 2 * TILE], dt.bfloat16,
                                         tag="el")
                    nc.vector._custom_dve(poly, out=el_t[:], in0=y[:],
                                          s0=PC1, s1=PC2)
                    parts = [el_t]
                else:
                    # exact drain: elu(y)+1 = relu(y) + min(e^y, 1); both
                    # terms go straight to the s-reduce (it accumulates)
                    t1 = bt_pool.tile([128, 2 * TILE], dt.bfloat16, tag="t1")
                    nc.scalar.activation(t1[:], y[:],
                                         mybir.ActivationFunctionType.Relu)
                    e_t = bt_pool.tile([128, 2 * TILE], dt.bfloat16, tag="e")
                    nc.scalar.activation(e_t[:], y[:],
                                         mybir.ActivationFunctionType.Exp)
                    t2 = bt_pool.tile([128, 2 * TILE], dt.bfloat16, tag="t2")
                    bt_mins[t] = [lambda t2=t2, e_t=e_t:
                                  nc.vector.tensor_scalar_min(t2[:], e_t[:],
                                                              1.0)]
                    parts = [t1, t2]

                pending.append((t, parts))
                if len(pending) >= FLUSH_AT:
                    flush_batch()

            while pending:
                flush_batch()

            s_sb = sout_pool.tile([128, TILE], dt.float32)
            nc.vector.tensor_copy(out=s_sb[0:64, 0:1], in_=warm_ps[0:64, 0:1])
            # final drain on ACT: it goes idle a few us before the last
            # s-matmuls retire, while DVE still has poly drains queued
            nc.scalar.activation(s_sb[:], psum_s[:],
                                 mybir.ActivationFunctionType.Copy)
            nc.scalar.dma_start(s_d[:], s_sb[:])

    nc.compile()
    _BASS_CACHE[key] = nc
    return nc


# ----------------------------------------------------------------------------
# Host-side wrapper
# ----------------------------------------------------------------------------

def kernel(instruction_batch, distribution, node_prop_similarities,
           relation_similarity, node_attrs, edge_attrs,
           W_node_props, W_edge, w_node_score, w_rel_score,
           node_indices, edge_batch_indices, edge_indices):
    from concourse.bass_utils import run_bass_kernel_spmd

    ib = np.asarray(instruction_batch, dtype=np.float32)
    dist = np.asarray(distribution, dtype=np.float32)
    sim = np.asarray(node_prop_similarities, dtype=np.float32)
    rsim = np.asarray(relation_similarity, dtype=np.float32)
    na = np.asarray(node_attrs, dtype=np.float32)
    ea = np.asarray(edge_attrs, dtype=np.float32)
    Wp = np.asarray(W_node_props, dtype=np.float32)
    We = np.asarray(W_edge, dtype=np.float32)
    wn = np.asarray(w_node_score, dtype=np.float32)
    wr = np.asarray(w_rel_score, dtype=np.float32)
    ni = np.asarray(node_indices).astype(np.int64)
    ebi = np.asarray(edge_batch_indices).astype(np.int64)
    ei = np.asarray(edge_indices).astype(np.int64)
    src, dst = ei[0], ei[1]

    B = ib.shape[0]
    N = na.shape[0]
    G = B // N_CORES  # graphs (slots) per core

    cn = np.bincount(ni, minlength=B)
    ce = np.bincount(ebi, minlength=B)
    nstart = np.concatenate([[0], np.cumsum(cn)])
    eperm = np.argsort(ebi, kind="stable")
    estart = np.concatenate([[0], np.cumsum(ce)])

    # ---- layout plan: rank graphs by edge count, slot j = ranks [8j, 8j+8)
    order = np.argsort(-ce, kind="stable")
    slot_graphs = order.reshape(G, N_CORES)          # [slot, dev] -> graph
    Ln = (-(-cn[slot_graphs].max(axis=1) // 4)) * 4  # per-slot node run len
    Le = (-(-ce[slot_graphs].max(axis=1) // 4)) * 4

    # Region order [edges slots 8-15 | nodes 0-15 | edges 0-7]: the first
    # third references only mats chunk 0, so only 1 MB of weights is on the
    # startup critical path; the rest streams during the first regions.
    MINP = 4  # LDWEIGHTS pipelines under even tiny matmuls; no snap needed

    def place(lens, o0):
        offs, lens2 = [], []
        o = int(o0)
        for ln in lens:
            ln = int(ln)
            r = o % TILE
            if r and TILE - r < MINP:
                o += TILE - r
            end = o + ln
            tail = end % TILE
            if end // TILE > o // TILE and 0 < tail < MINP:
                ln += MINP - tail
            offs.append(o)
            lens2.append(ln)
            o += ln
        return offs, lens2, o

    eoff_hi, Le2_hi, r1 = place([Le[j] for j in range(G // 2, G)], 0)
    noff, Ln2, r2 = place(Ln, r1)
    eoff_lo, Le2_lo, total = place([Le[j] for j in range(G // 2)], r2)
    eoff = eoff_lo + eoff_hi                # [slot] -> column offset
    Le2 = Le2_lo + Le2_hi
    n_tiles = -(-total // TILE)
    m_pad = n_tiles * TILE
    assert n_tiles <= 128, "s accumulator bank overflow"

    def u_of(j, typ):                       # weight index in use order
        if typ == 0:
            return 8 + j
        return 24 + j if j < G // 2 else j - G // 2

    runs = [(noff[j], Ln2[j], u_of(j, 0)) for j in range(G)] + \
           [(eoff[j], Le2[j], u_of(j, 1)) for j in range(G)]
    pieces = [[] for _ in range(n_tiles)]
    for (st, ln, u) in runs:
        if ln == 0:
            continue
        for t in range(st // TILE, (st + ln - 1) // TILE + 1):
            a = max(st, TILE * t) - TILE * t
            b = min(st + ln, TILE * (t + 1)) - TILE * t
            pieces[t].append((a, b, u))
    for p in pieces:
        p.sort()
    stypes = []
    for t in range(n_tiles):
        sr = []
        for (lo, hi, typ) in ((0, r1, 1), (r1, r2, 0), (r2, m_pad, 1)):
            a = max(lo, TILE * t) - TILE * t
            b = min(hi, TILE * (t + 1)) - TILE * t
            if a < b:
                sr.append((a, b, typ))
        stypes.append(tuple(sr))
    pieces = tuple(tuple(p) for p in pieces)
    stypes = tuple(stypes)

    # ---- item columns, transposed + bf16, packed per plan ----
    na_bf = na.astype(BF16)
    ea_bf = ea[eperm].astype(BF16)
    itemsv = np.zeros((N_CORES, 128, n_tiles, 2, TILE), dtype=BF16)

    def put(dev, col0, block):
        n = block.shape[0]
        bT = block.T.reshape(2, 128, n)  # [kc, p, n]
        j = np.arange(col0, col0 + n)
        tt, jj = j // TILE, j % TILE
        itemsv[dev][:, tt, 0, jj] = bT[0]
        itemsv[dev][:, tt, 1, jj] = bT[1]

    for j in range(G):
        for d in range(N_CORES):
            g = int(slot_graphs[j, d])
            put(d, int(noff[j]), na_bf[nstart[g]:nstart[g + 1]])
            put(d, int(eoff[j]), ea_bf[estart[g]:estart[g + 1]])
    items = itemsv.reshape(N_CORES, 128, 2 * m_pad)

    # ---- per-graph matrices A[k, d] (instr folded in), bf16 ----
    C = np.einsum("gp,pde->gde", sim, Wp)
    A_node = (C * ib[:, :, None]).transpose(0, 2, 1)           # [g, k, d]
    A_edge = (We[None, :, :] * ib[:, :, None]).transpose(0, 2, 1)
    A_all = np.empty((N_CORES, 2 * G, D, D), np.float32)       # [dev, u, k, d]
    for j in range(G):
        for d in range(N_CORES):
            g = int(slot_graphs[j, d])
            A_all[d, u_of(j, 0)] = A_node[g]
            A_all[d, u_of(j, 1)] = A_edge[g]
    # blob[p, ((u*2+kc)*2+dc)*128 + m] = A_u[kc*128+p][dc*128+m]
    Ar = A_all.reshape(N_CORES, 2 * G, 2, 128, 2, 128)  # dev,u,kc,p,dc,m
    mats = np.ascontiguousarray(Ar.transpose(0, 3, 1, 2, 4, 5)
                                ).reshape(N_CORES, 128, -1).astype(BF16)

    # ---- w tables: wtab[k, ((c*2+typ)*2+kc)*32+m] = w_typ[kc*128+k]*(m==c)
    wt = np.stack([wn, wr]).astype(np.float32)                  # [2, 256]
    eye = np.eye(32, dtype=np.float32)
    wtab = np.einsum("tk,cm->kctm", wt.reshape(2, 2, 128).reshape(4, 128), eye)
    wtab = np.ascontiguousarray(wtab.reshape(128, 32, 2, 2, 32)
                                ).reshape(128, 4 * 32 * 32).astype(BF16)

    # ---- run on 8 cores ----
    nc = _build_bass(n_tiles, pieces, stypes)
    in_maps = [{"items": items[d], "mats": mats[d], "wtab": wtab}
               for d in range(N_CORES)]
    res = run_bass_kernel_spmd(nc, in_maps, core_ids=list(range(N_CORES)))
    s_rows = np.stack([r["s_out"] for r in res.results])        # [8, 128, 512]

    # ---- unshard + finish on host ----
    sum_wn = float(wt[0].astype(BF16).astype(np.float32).sum())
    sum_wr = float(wt[1].astype(BF16).astype(np.float32).sum())
    state_logits = np.empty(N, np.float32)
    s_e = np.empty(ei.shape[1], np.float32)
    # tile t lives at psum row 32*(t%4) + (t//4)%32
    row_of_tile = np.array([32 * (t % 4) + (t // 4) % 32
                            for t in range(n_tiles)])
    flat = s_rows.reshape(N_CORES, -1)

    def gather(dev, o, ln):
        j = np.arange(o, o + ln)
        return flat[dev][row_of_tile[j // TILE] * TILE + (j % TILE)]

    for j in range(G):
        for d in range(N_CORES):
            g = int(slot_graphs[j, d])
            state_logits[nstart[g]:nstart[g + 1]] = \
                gather(d, int(noff[j]), int(cn[g])) - sum_wn
            s_e[estart[g]:estart[g + 1]] = \
                gather(d, int(eoff[j]), int(ce[g])) - sum_wr

    rel_logits = np.bincount(dst[eperm], weights=dist[src[eperm]] * s_e,
                             minlength=N).astype(np.float32)

    def seg_softmax(x):
        mx = np.maximum.reduceat(x, nstart[:-1])
        ex = np.exp(x - mx[ni])
        sm = np.add.reduceat(ex, nstart[:-1])
        return ex / sm[ni]

    r = rsim[ni]
    out = r * seg_softmax(rel_logits) + (1.0 - r) * seg_softmax(state_logits)
    return out.astype(np.float32)


# revision 32
# speedup vs baseline: 1.0042x; 1.0042x over previous
"""Trainium2 Bass kernel for nn_NSMCell (GNN message passing).

Strategy
--------
The reference output is only [N]: a per-graph blend of two segment softmaxes
over per-node scalars.  Both scalars are of the form

    s_i = sum_d w_d * elu( M_g[d, :] @ x_i )

where for "node items" M_g = (sim[g] . W_node_props) * instr[g] and x = node
attr, and for "edge items" M_g = W_edge * instr[g] and x = edge attr.  The
per-graph matrices are built on the host (they are tiny); the device streams
all item columns through matmuls + an elu drain + a weighted partition
reduce.  The edge-message scatter (index_add) collapses to a host-side
bincount of per-edge scalars, and the segment softmax + blend run on the
host over [N] values (negligible work).

Sharding: graphs are ranked by edge count and dealt round-robin so core d
gets slot-j graph rank 8j+d.  All 8 cores share one NEFF: per-slot run
lengths are the max over the 8 cores' graphs in that slot.  Items are
packed [edge slots 8-15 | all node runs | edge slots 0-7] so only the
first 1 MB weight chunk gates startup.

Device layout per 512-item tile (d on partitions, 2 chunks of 128 side by
side in one 2-bank PSUM tile):
  y[d, e]   = A_seg[k, d]^T @ xT[k, e]      2 matmuls per (piece, dc) -> PSUM

The elu(y)+1 drain is split across engines so neither ACT nor DVE is a
wall (any PSUM-reading pass runs at ~1 col/cycle with no 2x mode):
  - "poly" tiles (first N_A): one custom DVE op computes
        EL1 = min(relu(c1*y + c2)^4, relu(y) + 1)  ~= elu(y)+1
    in a single pass (max abs err ~0.03, distribution rms ~0.01; errors
    are averaged over 256 d's by the w-reduce so the softmax impact is
    ~0.3%).  Tiles are contiguous so most graphs see a homogeneous path.
  - "exact" tiles (rest): ACT Relu pass -> T1, ACT Exp pass -> E, then a
    fused scalar_tensor_tensor  EL1 = min(E, 1) + T1  on GPSIMD/DVE
    (bf16 SBUF, cheap).  Identity: elu(y)+1 = relu(y) + min(e^y, 1).

The s-reduce  s_row += (w (x) delta_c)^T @ EL1  uses M=32 matmuls which
waste 3/4 of the PE array; the 128x128 array is 16 independent 32x32
subarrays, so s-matmuls for 4 consecutive tiles are issued back-to-back
into 4 distinct 32-column groups (tile_position) and run concurrently:
tile t -> col group t%4, row (t//4)%32 selected by the (w (x) delta_c)
weight.  The s-bank is drained once at the end; the host subtracts
sum(w) to undo the +1.

Item DMAs alternate between the SP and Pool HWDGE queues plus the ACT
queue; one queue caps at ~175 GB/s, two sustain ~260 GB/s.
"""

import numpy as np
import ml_dtypes

BF16 = ml_dtypes.bfloat16
N_CORES = 8
D = 256
TILE = 512  # items per tile

# elu(y)+1 ~= min(relu(PC1*y + PC2)^4, relu(y)+1)  (minimax fit: unbiased
# enough that graphs mixing poly and exact tiles stay accurate)
PC1 = 0.20922107052487887
PC2 = 0.9984114123856602
# tile classes: "a" = single-pass DVE poly drain, "b" = exact ACT relu/exp
# pair.  Interleave so DVE and ACT run concurrently (DVE/ACT balance at
# a:b ~ 2100:1141 ~ 11:6), never adjacent (a b-tile holds its y PSUM for
# ~2.2us of ACT chain; clustering them stalls the 3-deep y pipeline).
_B_POS = {1, 4, 7, 10, 13, 15}  # within a period of 17
A_TAIL = 8  # last tiles forced to the short poly chain to shrink the tail


def _is_b_tile(t, n_tiles=1 << 30):
    return (t % 17) in _B_POS and t < n_tiles - A_TAIL


FLUSH_AT = 8      # s-matmul batch-of-4 issue slack (in tiles)


# ----------------------------------------------------------------------------
# Bass kernel builder (one NEFF shared by all cores)
# ----------------------------------------------------------------------------

_BASS_CACHE = {}


def _get_polyel1_op():
    """Register (once) a custom fused DVE op:

        out = min(relu(in0*s0 + s1)^4, relu(in0) + 1)

    With in0 = y this computes elu(y)+1 to ~0.03 abs err in one VectorE
    pass straight out of PSUM, replacing the ACT exp + DVE min pair."""
    from concourse import dve_ops
    from concourse.dve_spec import (Spec, Src0, C0, C1, One, relu, sq, minn,
                                    lower, _has_src1)
    from concourse.dve_uop import DveOpSpec

    for o in dve_ops.OPS:
        if o.name == "POLYEL1_ANT":
            return o

    def ref(in0, in1, s0, s1, imm2):
        x = in0.astype(np.float32)
        g = np.maximum(x * s0 + s1, 0.0) ** 4
        return np.minimum(g, np.maximum(x, 0.0) + 1.0).astype(np.float32)

    body = minn(sq(sq(relu(Src0 * C0 + C1))), relu(Src0) + One)
    spec = Spec(body=body, reference=ref)
    row = dve_ops._CUSTOM_DVE_ROW_BASE + len(dve_ops.OPS)
    shas = {}
    for ver in ("v3", "v4"):
        uops = lower(spec, ver=ver)
        shas[ver] = DveOpSpec(name="POLYEL1_ANT", opcode=row, uops=uops,
                              rd1_en=_has_src1(spec)).sha(ver)
    op = dve_ops.DveOp("POLYEL1_ANT", spec, subdim=False, uops_sha=shas)
    dve_ops.OPS.append(op)
    dve_ops.CUSTOM_DVE_SPECS[op.name] = op.spec
    dve_ops._SUB_OPCODE_FOR_NAME[op.name] = row
    return op


def _build_bass(n_tiles, pieces, stypes):
    """Build the Tile/Bass program.

    n_tiles: number of 512-item tiles per core
    pieces:  per tile, tuple of (a, b, u): y-matmul column range [a, b) using
             per-graph matrix u (u = slot for nodes, 16 + slot for edges)
    stypes:  per tile, tuple of (ra, rb, typ): s-reduce column ranges by
             item type (0 = node -> w_node, 1 = edge -> w_rel)
    """
    key = (n_tiles, pieces, stypes, tuple(sorted(_B_POS)), A_TAIL)
    if key in _BASS_CACHE:
        return _BASS_CACHE[key]

    import concourse.mybir as mybir
    import concourse.tile as tile
    from concourse import bacc

    dt = mybir.dt
    n_seg = 32  # 16 slots x (node, edge), u-ordered: nodes 0-15, edges 16-31
    assert n_tiles <= 128

    poly = _get_polyel1_op()
    nc = bacc.Bacc("TRN2", target_bir_lowering=False)
    m_pad = n_tiles * TILE
    items_d = nc.dram_tensor("items", [128, 2 * m_pad], dt.bfloat16,
                             kind="ExternalInput")
    mats_d = nc.dram_tensor("mats", [128, n_seg * 2 * 2 * 128], dt.bfloat16,
                            kind="ExternalInput")
    wtab_d = nc.dram_tensor("wtab", [128, 2 * 2 * 32 * 32], dt.bfloat16,
                            kind="ExternalInput")
    s_d = nc.dram_tensor("s_out", [128, TILE], dt.float32,
                         kind="ExternalOutput")

    with tile.TileContext(nc) as tc:
        with (
            tc.tile_pool(name="const", bufs=1) as const_pool,
            tc.tile_pool(name="items", bufs=16) as item_pool,
            tc.tile_pool(name="psum_y", bufs=3, space="PSUM") as ypool,
            tc.tile_pool(name="psum_s", bufs=1, space="PSUM") as spool,
            tc.tile_pool(name="elu", bufs=10) as elu_pool,
            tc.tile_pool(name="bt", bufs=6) as bt_pool,
            tc.tile_pool(name="sout", bufs=1) as sout_pool,
            tc.tile_pool(name="warm", bufs=1, space="PSUM") as warm_pool,
        ):
            # Consts: one pool tile per chunk so the chunk DMAs carry no
            # same-tile WAW deps (they'd serialize otherwise).
            MCH = 8  # u-slots per mats chunk
            mats_sbs = [const_pool.tile([128, MCH * 512], dt.bfloat16,
                                        name=f"matsb{i}", tag=f"mats{i}")
                        for i in range(n_seg // MCH)]
            wtab_sbs = [const_pool.tile([128, 8 * 128], dt.bfloat16,
                                        name=f"wtabb{i}", tag=f"wtab{i}")
                        for i in range(4)]

            def load_mats(ch, eng=None):
                sl = slice(ch * MCH * 512, (ch + 1) * MCH * 512)
                (eng or nc.scalar).dma_start(mats_sbs[ch][:], mats_d[:, sl])

            def load_mats_q(ch, q, eng):
                # one quarter of a mats chunk - slipped between item DMAs
                w = MCH * 512 // 4
                sl = slice(ch * MCH * 512 + q * w, ch * MCH * 512 + (q + 1) * w)
                eng.dma_start(mats_sbs[ch][:, q * w:(q + 1) * w],
                              mats_d[:, sl])

            def load_wtab(ch):
                sl = slice(ch * 8 * 128, (ch + 1) * 8 * 128)
                nc.scalar.dma_start(wtab_sbs[ch][:, :], wtab_d[:, sl])

            # HAM pre-warm: keep PE busy during the DMA preamble so real
            # matmuls start at 2.4 GHz instead of ramping from 1.2 GHz.
            # gpsimd issues its first instruction ~1us before the other
            # engines, so the warm block starts earliest off its memset;
            # ~75 warm matmuls bridge until the first item lands so the HAM
            # un-throttles right as real matmuls begin.
            warm_sb = const_pool.tile([128, 64], dt.bfloat16)
            nc.gpsimd.memset(warm_sb[:], 0)
            warm_ps = warm_pool.tile([128, 64], dt.float32)
            for _ in range(75):
                nc.tensor.matmul(warm_ps[0:64, :], warm_sb[:], warm_sb[:],
                                 start=True, stop=True, skip_group_check=True)

            # Startup: items stream on the SP+Pool queues from tile 0 while
            # the ACT queue carries mats chunk 0 (quartered so the first
            # y-matmuls release as soon as their slice lands) and wtab 0.
            # Everything else is deferred near its first use so the three
            # queues are item-dedicated once the pipeline is hot.
            first_use = {ch: n_tiles for ch in range(4)}
            for t in range(n_tiles):
                for (_, _, u) in pieces[t]:
                    first_use[u // MCH] = min(first_use[u // MCH], t)
            const_sched = {}

            def sched(tq, fn):
                const_sched.setdefault(tq, []).append(fn)

            # mats ch0 in eighths: the first eighth (the weights of tile 0)
            # leads the SP queue ahead of its first item; the rest stream on
            # the ACT queue which carries no items until tile 11.
            def load_mats_8(ch, q, eng):
                w = MCH * 512 // 8
                sl = slice(ch * MCH * 512 + q * w, ch * MCH * 512 + (q + 1) * w)
                eng.dma_start(mats_sbs[ch][:, q * w:(q + 1) * w], mats_d[:, sl])

            sched(0, lambda: load_mats_8(0, 0, nc.sync))
            for q in range(1, 8):
                sched(q - 1, lambda q=q: load_mats_8(0, q, nc.scalar))
            sched(7, lambda: load_wtab(0))

            # all on the ACT queue: it only carries 1 in 3 item tiles after
            # joining the rotation, so const quarters slot into its gaps
            # without stealing item bandwidth from the SP/Pool queues
            for ch, t0 in ((1, 8), (2, 16), (3, 24)):
                t0 = min(t0, max(2, first_use[ch] - 12))
                for q in range(4):
                    sched(t0 + 2 * q, lambda ch=ch, q=q:
                          load_mats_q(ch, q, nc.scalar))
            # wtab chunk g is first used at tile 32g (s-row c = (t//4)%32)
            sched(20, lambda: load_wtab(1))
            if n_tiles > 64:
                sched(48, lambda: load_wtab(2))
            if n_tiles > 96:
                sched(72, lambda: load_wtab(3))
            psum_s = spool.tile([128, TILE], dt.float32)

            def mat_sl(u, kc, dc):
                ch, s = divmod(u, MCH)
                off = ((s * 2 + kc) * 2 + dc) * 128
                return mats_sbs[ch][:, off:off + 128]

            def w_sl(typ, kc, c):
                ch, cc = divmod(c, 8)
                off = ((cc * 2 + typ) * 2 + kc) * 32
                return wtab_sbs[ch][:, off:off + 32]

            # s-matmul col-group schedule: tile t -> col group g = t%4 (PSUM
            # partitions 32g..32g+32), row within group c = (t//4)%32 picked
            # by the (w (x) delta_c) weight.  Batches of 4 consecutive tiles
            # issue back-to-back so the 4 col groups compute concurrently.
            strip_started = set()
            strip_last = {g: max(range(g, n_tiles, 4)) for g in range(4)
                          if g < n_tiles}
            pending = []  # (t, [el parts to accumulate])
            bt_mins = {}  # t -> deferred min(e,1) emitters

            def flush_batch():
                batch = pending[:4]
                del pending[:4]
                # deferred b-tile mins: they only feed this batch's s-matmuls
                # so they must not sit in DVE's FIFO ahead of the PSUM-
                # critical poly drains
                for (t, parts) in batch:
                    for fn in bt_mins.pop(t, ()):
                        fn()
                for kc in range(2):
                    for (t, parts) in batch:
                        g, c = t % 4, (t // 4) % 32
                        out_rows = psum_s[32 * g:32 * g + 32, :]
                        tp = (0, 32 * g)
                        srs = stypes[t]
                        for pi, el_t in enumerate(parts):
                            for ri, (ra, rb, typ) in enumerate(srs):
                                st = g not in strip_started
                                strip_started.add(g)
                                stop = (t == strip_last[g] and kc == 1
                                        and pi == len(parts) - 1
                                        and ri == len(srs) - 1)
                                nc.tensor.matmul(
                                    out_rows[:, ra:rb], w_sl(typ, kc, c),
                                    el_t[:, kc * TILE + ra:kc * TILE + rb],
                                    start=st, stop=stop,
                                    tile_position=tp, skip_group_check=True)

            for t in range(n_tiles):
                for fn in const_sched.get(t, ()):
                    fn()
                x2 = item_pool.tile([128, 2 * TILE], dt.bfloat16, tag="x")
                xoff = 0
                # early: Pool gets the even tiles (its queue is otherwise
                # empty; SP's first item rides behind mats q0)
                if t < 9:
                    eng = nc.gpsimd if t % 2 == 0 else nc.sync
                else:
                    eng = (nc.sync, nc.gpsimd, nc.scalar)[t % 3]
                eng.dma_start(
                    x2[:], items_d[:, t * 2 * TILE:(t + 1) * 2 * TILE])

                # both d-chunks side by side in one 2-bank PSUM tile.
                # Snake the (dc, kc) order across tiles: within a graph run
                # the weights repeat tile-to-tile, so the first matmul of
                # tile t reuses the weight the last matmul of t-1 loaded and
                # doesn't wait on a fresh LDWEIGHTS.
                y = ypool.tile([128, 2 * TILE], dt.float32, tag="y")
                n_p = len(pieces[t])
                combos = [(0, 0), (0, 1), (1, 1), (1, 0)]  # (dc, kc)
                if t % 2:
                    combos = combos[::-1]
                for pi, (a, b, u) in enumerate(pieces[t]):
                    for ci, (dc, kc) in enumerate(combos):
                        # ci 0/2 are each dc-half's first write: start=True
                        # marks that half's PSUM bank pending-zero
                        ysl = y[:, dc * TILE + a:dc * TILE + b]
                        nc.tensor.matmul(
                            ysl, mat_sl(u, kc, dc),
                            x2[:, xoff + kc * TILE + a:xoff + kc * TILE + b],
                            start=(pi == 0 and ci in (0, 2)),
                            stop=(pi == n_p - 1 and ci in (1, 3)),
                            skip_group_check=True)

                if not _is_b_tile(t, n_tiles):
                    # single-pass poly drain on DVE
                    el_t = elu_pool.tile([128, 2 * TILE], dt.bfloat16,
                                         tag="el")
                    nc.vector._custom_dve(poly, out=el_t[:], in0=y[:],
                                          s0=PC1, s1=PC2)
                    parts = [el_t]
                else:
                    # exact drain: elu(y)+1 = relu(y) + min(e^y, 1); both
                    # terms go straight to the s-reduce (it accumulates)
                    t1 = bt_pool.tile([128, 2 * TILE], dt.bfloat16, tag="t1")
                    nc.scalar.activation(t1[:], y[:],
                                         mybir.ActivationFunctionType.Relu)
                    e_t = bt_pool.tile([128, 2 * TILE], dt.bfloat16, tag="e")
                    nc.scalar.activation(e_t[:], y[:],
                                         mybir.ActivationFunctionType.Exp)
                    t2 = bt_pool.tile([128, 2 * TILE], dt.bfloat16, tag="t2")
                    nc.vector.tensor_scalar_min(t2[:], e_t[:], 1.0)
                    parts = [t1, t2]

                pending.append((t, parts))
                if len(pending) >= FLUSH_AT:
                    flush_batch()

            while pending:
                flush_batch()

            s_sb = sout_pool.tile([128, TILE], dt.float32)
            nc.vector.tensor_copy(out=s_sb[0:64, 0:1], in_=warm_ps[0:64, 0:1])
            # final drain on ACT: it goes idle a few us before the last
            # s-matmuls retire, while DVE still has poly drains queued
            nc.scalar.activation(s_sb[:], psum_s[:],
                                 mybir.ActivationFunctionType.Copy)
            nc.scalar.dma_start(s_d[:], s_sb[:])

    nc.compile()
    _BASS_CACHE[key] = nc
    return nc


# ----------------------------------------------------------------------------
# Host-side wrapper
# ----------------------------------------------------------------------------

def kernel(instruction_batch, distribution, node_prop_similarities,
           relation_similarity, node_attrs, edge_attrs,
           W_node_props, W_edge, w_node_score, w_rel_score,
           node_indices, edge_batch_indices, edge_indices):
    from concourse.bass_utils import run_bass_kernel_spmd

    ib = np.asarray(instruction_batch, dtype=np.float32)
    dist = np.asarray(distribution, dtype=np.float32)
    sim = np.asarray(node_prop_similarities, dtype=np.float32)
    rsim = np.asarray(relation_similarity, dtype=np.float32)
    na = np.asarray(node_attrs, dtype=np.float32)
    ea = np.asarray(edge_attrs, dtype=np.float32)
    Wp = np.asarray(W_node_props, dtype=np.float32)
    We = np.asarray(W_edge, dtype=np.float32)
    wn = np.asarray(w_node_score, dtype=np.float32)
    wr = np.asarray(w_rel_score, dtype=np.float32)
    ni = np.asarray(node_indices).astype(np.int64)
    ebi = np.asarray(edge_batch_indices).astype(np.int64)
    ei = np.asarray(edge_indices).astype(np.int64)
    src, dst = ei[0], ei[1]

    B = ib.shape[0]
    N = na.shape[0]
    G = B // N_CORES  # graphs (slots) per core

    cn = np.bincount(ni, minlength=B)
    ce = np.bincount(ebi, minlength=B)
    nstart = np.concatenate([[0], np.cumsum(cn)])
    eperm = np.argsort(ebi, kind="stable")
    estart = np.concatenate([[0], np.cumsum(ce)])

    # ---- layout plan: rank graphs by edge count, slot j = ranks [8j, 8j+8)
    order = np.argsort(-ce, kind="stable")
    slot_graphs = order.reshape(G, N_CORES)          # [slot, dev] -> graph
    Ln = (-(-cn[slot_graphs].max(axis=1) // 4)) * 4  # per-slot node run len
    Le = (-(-ce[slot_graphs].max(axis=1) // 4)) * 4

    # Region order [edges slots 8-15 | nodes 0-15 | edges 0-7]: the first
    # third references only mats chunk 0, so only 1 MB of weights is on the
    # startup critical path; the rest streams during the first regions.
    MINP = 4  # LDWEIGHTS pipelines under even tiny matmuls; no snap needed

    def place(lens, o0):
        offs, lens2 = [], []
        o = int(o0)
        for ln in lens:
            ln = int(ln)
            r = o % TILE
            if r and TILE - r < MINP:
                o += TILE - r
            end = o + ln
            tail = end % TILE
            if end // TILE > o // TILE and 0 < tail < MINP:
                ln += MINP - tail
            offs.append(o)
            lens2.append(ln)
            o += ln
        return offs, lens2, o

    eoff_hi, Le2_hi, r1 = place([Le[j] for j in range(G // 2, G)], 0)
    noff, Ln2, r2 = place(Ln, r1)
    eoff_lo, Le2_lo, total = place([Le[j] for j in range(G // 2)], r2)
    eoff = eoff_lo + eoff_hi                # [slot] -> column offset
    Le2 = Le2_lo + Le2_hi
    n_tiles = -(-total // TILE)
    m_pad = n_tiles * TILE
    assert n_tiles <= 128, "s accumulator bank overflow"

    def u_of(j, typ):                       # weight index in use order
        if typ == 0:
            return 8 + j
        return 24 + j if j < G // 2 else j - G // 2

    runs = [(noff[j], Ln2[j], u_of(j, 0)) for j in range(G)] + \
           [(eoff[j], Le2[j], u_of(j, 1)) for j in range(G)]
    pieces = [[] for _ in range(n_tiles)]
    for (st, ln, u) in runs:
        if ln == 0:
            continue
        for t in range(st // TILE, (st + ln - 1) // TILE + 1):
            a = max(st, TILE * t) - TILE * t
            b = min(st + ln, TILE * (t + 1)) - TILE * t
            pieces[t].append((a, b, u))
    for p in pieces:
        p.sort()
    stypes = []
    for t in range(n_tiles):
        sr = []
        for (lo, hi, typ) in ((0, r1, 1), (r1, r2, 0), (r2, m_pad, 1)):
            a = max(lo, TILE * t) - TILE * t
            b = min(hi, TILE * (t + 1)) - TILE * t
            if a < b:
                sr.append((a, b, typ))
        stypes.append(tuple(sr))
    pieces = tuple(tuple(p) for p in pieces)
    stypes = tuple(stypes)

    # ---- item columns, transposed + bf16, packed per plan ----
    na_bf = na.astype(BF16)
    ea_bf = ea[eperm].astype(BF16)
    itemsv = np.zeros((N_CORES, 128, n_tiles, 2, TILE), dtype=BF16)

    def put(dev, col0, block):
        n = block.shape[0]
        bT = block.T.reshape(2, 128, n)  # [kc, p, n]
        j = np.arange(col0, col0 + n)
        tt, jj = j // TILE, j % TILE
        itemsv[dev][:, tt, 0, jj] = bT[0]
        itemsv[dev][:, tt, 1, jj] = bT[1]

    for j in range(G):
        for d in range(N_CORES):
            g = int(slot_graphs[j, d])
            put(d, int(noff[j]), na_bf[nstart[g]:nstart[g + 1]])
            put(d, int(eoff[j]), ea_bf[estart[g]:estart[g + 1]])
    items = itemsv.reshape(N_CORES, 128, 2 * m_pad)

    # ---- per-graph matrices A[k, d] (instr folded in), bf16 ----
    C = np.einsum("gp,pde->gde", sim, Wp)
    A_node = (C * ib[:, :, None]).transpose(0, 2, 1)           # [g, k, d]
    A_edge = (We[None, :, :] * ib[:, :, None]).transpose(0, 2, 1)
    A_all = np.empty((N_CORES, 2 * G, D, D), np.float32)       # [dev, u, k, d]
    for j in range(G):
        for d in range(N_CORES):
            g = int(slot_graphs[j, d])
            A_all[d, u_of(j, 0)] = A_node[g]
            A_all[d, u_of(j, 1)] = A_edge[g]
    # blob[p, ((u*2+kc)*2+dc)*128 + m] = A_u[kc*128+p][dc*128+m]
    Ar = A_all.reshape(N_CORES, 2 * G, 2, 128, 2, 128)  # dev,u,kc,p,dc,m
    mats = np.ascontiguousarray(Ar.transpose(0, 3, 1, 2, 4, 5)
                                ).reshape(N_CORES, 128, -1).astype(BF16)

    # ---- w tables: wtab[k, ((c*2+typ)*2+kc)*32+m] = w_typ[kc*128+k]*(m==c)
    wt = np.stack([wn, wr]).astype(np.float32)                  # [2, 256]
    eye = np.eye(32, dtype=np.float32)
    wtab = np.einsum("tk,cm->kctm", wt.reshape(2, 2, 128).reshape(4, 128), eye)
    wtab = np.ascontiguousarray(wtab.reshape(128, 32, 2, 2, 32)
                                ).reshape(128, 4 * 32 * 32).astype(BF16)

    # ---- run on 8 cores ----
    nc = _build_bass(n_tiles, pieces, stypes)
    in_maps = [{"items": items[d], "mats": mats[d], "wtab": wtab}
               for d in range(N_CORES)]
    res = run_bass_kernel_spmd(nc, in_maps, core_ids=list(range(N_CORES)))
    s_rows = np.stack([r["s_out"] for r in res.results])        # [8, 128, 512]

    # ---- unshard + finish on host ----
    sum_wn = float(wt[0].astype(BF16).astype(np.float32).sum())
    sum_wr = float(wt[1].astype(BF16).astype(np.float32).sum())
    state_logits = np.empty(N, np.float32)
    s_e = np.empty(ei.shape[1], np.float32)
    # tile t lives at psum row 32*(t%4) + (t//4)%32
    row_of_tile = np.array([32 * (t % 4) + (t // 4) % 32
                            for t in range(n_tiles)])
    flat = s_rows.reshape(N_CORES, -1)

    def gather(dev, o, ln):
        j = np.arange(o, o + ln)
        return flat[dev][row_of_tile[j // TILE] * TILE + (j % TILE)]

    for j in range(G):
        for d in range(N_CORES):
            g = int(slot_graphs[j, d])
            state_logits[nstart[g]:nstart[g + 1]] = \
                gather(d, int(noff[j]), int(cn[g])) - sum_wn
            s_e[estart[g]:estart[g + 1]] = \
                gather(d, int(eoff[j]), int(ce[g])) - sum_wr

    rel_logits = np.bincount(dst[eperm], weights=dist[src[eperm]] * s_e,
                             minlength=N).astype(np.float32)

    def seg_softmax(x):
        mx = np.maximum.reduceat(x, nstart[:-1])
        ex = np.exp(x - mx[ni])
        sm = np.add.reduceat(ex, nstart[:-1])
        return ex / sm[ni]

    r = rsim[ni]
    out = r * seg_softmax(rel_logits) + (1.0 - r) * seg_softmax(state_logits)
    return out.astype(np.float32)


# revision 33
# speedup vs baseline: 1.0155x; 1.0112x over previous
"""Trainium2 Bass kernel for nn_NSMCell (GNN message passing).

Strategy
--------
The reference output is only [N]: a per-graph blend of two segment softmaxes
over per-node scalars.  Both scalars are of the form

    s_i = sum_d w_d * elu( M_g[d, :] @ x_i )

where for "node items" M_g = (sim[g] . W_node_props) * instr[g] and x = node
attr, and for "edge items" M_g = W_edge * instr[g] and x = edge attr.  The
per-graph matrices are built on the host (they are tiny); the device streams
all item columns through matmuls + an elu drain + a weighted partition
reduce.  The edge-message scatter (index_add) collapses to a host-side
bincount of per-edge scalars, and the segment softmax + blend run on the
host over [N] values (negligible work).

Sharding: graphs are ranked by edge count and dealt round-robin so core d
gets slot-j graph rank 8j+d.  All 8 cores share one NEFF: per-slot run
lengths are the max over the 8 cores' graphs in that slot.  Items are
packed [edge slots 8-15 | all node runs | edge slots 0-7] so only the
first 1 MB weight chunk gates startup.

Device layout per 512-item tile (d on partitions, 2 chunks of 128 side by
side in one 2-bank PSUM tile):
  y[d, e]   = A_seg[k, d]^T @ xT[k, e]      2 matmuls per (piece, dc) -> PSUM

The elu(y)+1 drain is split across engines so neither ACT nor DVE is a
wall (any PSUM-reading pass runs at ~1 col/cycle with no 2x mode):
  - "poly" tiles (first N_A): one custom DVE op computes
        EL1 = min(relu(c1*y + c2)^4, relu(y) + 1)  ~= elu(y)+1
    in a single pass (max abs err ~0.03, distribution rms ~0.01; errors
    are averaged over 256 d's by the w-reduce so the softmax impact is
    ~0.3%).  Tiles are contiguous so most graphs see a homogeneous path.
  - "exact" tiles (rest): ACT Relu pass -> T1, ACT Exp pass -> E, then a
    fused scalar_tensor_tensor  EL1 = min(E, 1) + T1  on GPSIMD/DVE
    (bf16 SBUF, cheap).  Identity: elu(y)+1 = relu(y) + min(e^y, 1).

The s-reduce  s_row += (w (x) delta_c)^T @ EL1  uses M=32 matmuls which
waste 3/4 of the PE array; the 128x128 array is 16 independent 32x32
subarrays, so s-matmuls for 4 consecutive tiles are issued back-to-back
into 4 distinct 32-column groups (tile_position) and run concurrently:
tile t -> col group t%4, row (t//4)%32 selected by the (w (x) delta_c)
weight.  The s-bank is drained once at the end; the host subtracts
sum(w) to undo the +1.

Item DMAs alternate between the SP and Pool HWDGE queues plus the ACT
queue; one queue caps at ~175 GB/s, two sustain ~260 GB/s.
"""

import numpy as np
import ml_dtypes

BF16 = ml_dtypes.bfloat16
N_CORES = 8
D = 256
TILE = 512  # items per tile

# elu(y)+1 ~= min(relu(PC1*y + PC2)^4, relu(y)+1)  (minimax fit: unbiased
# enough that graphs mixing poly and exact tiles stay accurate)
PC1 = 0.20922107052487887
PC2 = 0.9984114123856602
# tile classes: "a" = single-pass DVE poly drain, "b" = exact ACT relu/exp
# pair.  Interleave so DVE and ACT run concurrently (DVE/ACT balance at
# a:b ~ 2100:1141 ~ 11:6), never adjacent (a b-tile holds its y PSUM for
# ~2.2us of ACT chain; clustering them stalls the 3-deep y pipeline).
_B_POS = {1, 4, 7, 10, 13, 15}  # within a period of 17
A_TAIL = 8  # last tiles forced to the short poly chain to shrink the tail


def _is_b_tile(t, n_tiles=1 << 30):
    return (t % 17) in _B_POS and t < n_tiles - A_TAIL


FLUSH_AT = 8      # s-matmul batch-of-4 issue slack (in tiles)


# ----------------------------------------------------------------------------
# Bass kernel builder (one NEFF shared by all cores)
# ----------------------------------------------------------------------------

_BASS_CACHE = {}


def _get_polyel1_op():
    """Register (once) a custom fused DVE op:

        out = min(relu(in0*s0 + s1)^4, relu(in0) + 1)

    With in0 = y this computes elu(y)+1 to ~0.03 abs err in one VectorE
    pass straight out of PSUM, replacing the ACT exp + DVE min pair."""
    from concourse import dve_ops
    from concourse.dve_spec import (Spec, Src0, C0, C1, One, relu, sq, minn,
                                    lower, _has_src1)
    from concourse.dve_uop import DveOpSpec

    for o in dve_ops.OPS:
        if o.name == "POLYEL1_ANT":
            return o

    def ref(in0, in1, s0, s1, imm2):
        x = in0.astype(np.float32)
        g = np.maximum(x * s0 + s1, 0.0) ** 4
        return np.minimum(g, np.maximum(x, 0.0) + 1.0).astype(np.float32)

    body = minn(sq(sq(relu(Src0 * C0 + C1))), relu(Src0) + One)
    spec = Spec(body=body, reference=ref)
    row = dve_ops._CUSTOM_DVE_ROW_BASE + len(dve_ops.OPS)
    shas = {}
    for ver in ("v3", "v4"):
        uops = lower(spec, ver=ver)
        shas[ver] = DveOpSpec(name="POLYEL1_ANT", opcode=row, uops=uops,
                              rd1_en=_has_src1(spec)).sha(ver)
    op = dve_ops.DveOp("POLYEL1_ANT", spec, subdim=False, uops_sha=shas)
    dve_ops.OPS.append(op)
    dve_ops.CUSTOM_DVE_SPECS[op.name] = op.spec
    dve_ops._SUB_OPCODE_FOR_NAME[op.name] = row
    return op


def _build_bass(n_tiles, pieces, stypes):
    """Build the Tile/Bass program.

    n_tiles: number of 512-item tiles per core
    pieces:  per tile, tuple of (a, b, u): y-matmul column range [a, b) using
             per-graph matrix u (u = slot for nodes, 16 + slot for edges)
    stypes:  per tile, tuple of (ra, rb, typ): s-reduce column ranges by
             item type (0 = node -> w_node, 1 = edge -> w_rel)
    """
    key = (n_tiles, pieces, stypes, tuple(sorted(_B_POS)), A_TAIL)
    if key in _BASS_CACHE:
        return _BASS_CACHE[key]

    import concourse.mybir as mybir
    import concourse.tile as tile
    from concourse import bacc

    dt = mybir.dt
    n_seg = 32  # 16 slots x (node, edge), u-ordered: nodes 0-15, edges 16-31
    assert n_tiles <= 128

    poly = _get_polyel1_op()
    nc = bacc.Bacc("TRN2", target_bir_lowering=False)
    m_pad = n_tiles * TILE
    items_d = nc.dram_tensor("items", [128, 2 * m_pad], dt.bfloat16,
                             kind="ExternalInput")
    mats_d = nc.dram_tensor("mats", [128, n_seg * 2 * 2 * 128], dt.bfloat16,
                            kind="ExternalInput")
    wtab_d = nc.dram_tensor("wtab", [128, 2 * 2 * 32 * 32], dt.bfloat16,
                            kind="ExternalInput")
    s_d = nc.dram_tensor("s_out", [128, TILE], dt.float32,
                         kind="ExternalOutput")

    with tile.TileContext(nc) as tc:
        with (
            tc.tile_pool(name="const", bufs=1) as const_pool,
            tc.tile_pool(name="items", bufs=16) as item_pool,
            tc.tile_pool(name="psum_y", bufs=3, space="PSUM") as ypool,
            tc.tile_pool(name="psum_s", bufs=1, space="PSUM") as spool,
            tc.tile_pool(name="elu", bufs=10) as elu_pool,
            tc.tile_pool(name="bt", bufs=6) as bt_pool,
            tc.tile_pool(name="sout", bufs=1) as sout_pool,
            tc.tile_pool(name="warm", bufs=1, space="PSUM") as warm_pool,
        ):
            # Consts: one pool tile per chunk so the chunk DMAs carry no
            # same-tile WAW deps (they'd serialize otherwise).
            MCH = 8  # u-slots per mats chunk
            mats_sbs = [const_pool.tile([128, MCH * 512], dt.bfloat16,
                                        name=f"matsb{i}", tag=f"mats{i}")
                        for i in range(n_seg // MCH)]
            wtab_sbs = [const_pool.tile([128, 8 * 128], dt.bfloat16,
                                        name=f"wtabb{i}", tag=f"wtab{i}")
                        for i in range(4)]

            def load_mats(ch, eng=None):
                sl = slice(ch * MCH * 512, (ch + 1) * MCH * 512)
                (eng or nc.scalar).dma_start(mats_sbs[ch][:], mats_d[:, sl])

            def load_mats_q(ch, q, eng):
                # one quarter of a mats chunk - slipped between item DMAs
                w = MCH * 512 // 4
                sl = slice(ch * MCH * 512 + q * w, ch * MCH * 512 + (q + 1) * w)
                eng.dma_start(mats_sbs[ch][:, q * w:(q + 1) * w],
                              mats_d[:, sl])

            def load_wtab(ch):
                sl = slice(ch * 8 * 128, (ch + 1) * 8 * 128)
                nc.scalar.dma_start(wtab_sbs[ch][:, :], wtab_d[:, sl])

            # HAM pre-warm: keep PE busy during the DMA preamble so real
            # matmuls start at 2.4 GHz instead of ramping from 1.2 GHz.
            # gpsimd issues its first instruction ~1us before the other
            # engines, so the warm block starts earliest off its memset;
            # ~75 warm matmuls bridge until the first item lands so the HAM
            # un-throttles right as real matmuls begin.
            warm_sb = const_pool.tile([128, 64], dt.bfloat16)
            nc.gpsimd.memset(warm_sb[:], 0)
            warm_ps = warm_pool.tile([128, 64], dt.float32)
            for _ in range(75):
                nc.tensor.matmul(warm_ps[0:64, :], warm_sb[:], warm_sb[:],
                                 start=True, stop=True, skip_group_check=True)

            # Startup: items stream on the SP+Pool queues from tile 0 while
            # the ACT queue carries mats chunk 0 (quartered so the first
            # y-matmuls release as soon as their slice lands) and wtab 0.
            # Everything else is deferred near its first use so the three
            # queues are item-dedicated once the pipeline is hot.
            first_use = {ch: n_tiles for ch in range(4)}
            for t in range(n_tiles):
                for (_, _, u) in pieces[t]:
                    first_use[u // MCH] = min(first_use[u // MCH], t)
            const_sched = {}

            def sched(tq, fn):
                const_sched.setdefault(tq, []).append(fn)

            # mats q0 leads the SP queue (ahead of its first item) so tile
            # 0's weights land with the first items; q1-q3 stream on the
            # ACT queue which carries no items until tile 11.
            sched(0, lambda: load_mats_q(0, 0, nc.sync))
            for q in (1, 2, 3):
                sched(2 * q - 1, lambda q=q: load_mats_q(0, q, nc.scalar))
            sched(7, lambda: load_wtab(0))

            # all on the ACT queue: it only carries 1 in 3 item tiles after
            # joining the rotation, so const quarters slot into its gaps
            # without stealing item bandwidth from the SP/Pool queues
            for ch, t0 in ((1, 8), (2, 16), (3, 24)):
                t0 = min(t0, max(2, first_use[ch] - 12))
                for q in range(4):
                    sched(t0 + 2 * q, lambda ch=ch, q=q:
                          load_mats_q(ch, q, nc.scalar))
            # wtab chunk g is first used at tile 32g (s-row c = (t//4)%32)
            sched(20, lambda: load_wtab(1))
            if n_tiles > 64:
                sched(48, lambda: load_wtab(2))
            if n_tiles > 96:
                sched(72, lambda: load_wtab(3))
            psum_s = spool.tile([128, TILE], dt.float32)

            def mat_sl(u, kc, dc):
                ch, s = divmod(u, MCH)
                off = ((s * 2 + kc) * 2 + dc) * 128
                return mats_sbs[ch][:, off:off + 128]

            def w_sl(typ, kc, c):
                ch, cc = divmod(c, 8)
                off = ((cc * 2 + typ) * 2 + kc) * 32
                return wtab_sbs[ch][:, off:off + 32]

            # s-matmul col-group schedule: tile t -> col group g = t%4 (PSUM
            # partitions 32g..32g+32), row within group c = (t//4)%32 picked
            # by the (w (x) delta_c) weight.  Batches of 4 consecutive tiles
            # issue back-to-back so the 4 col groups compute concurrently.
            strip_started = set()
            strip_last = {g: max(range(g, n_tiles, 4)) for g in range(4)
                          if g < n_tiles}
            pending = []  # (t, [el parts to accumulate])
            bt_mins = {}  # t -> deferred min(e,1) emitters

            def flush_batch():
                batch = pending[:4]
                del pending[:4]
                # deferred b-tile mins: they only feed this batch's s-matmuls
                # so they must not sit in DVE's FIFO ahead of the PSUM-
                # critical poly drains
                for (t, parts) in batch:
                    for fn in bt_mins.pop(t, ()):
                        fn()
                for kc in range(2):
                    for (t, parts) in batch:
                        g, c = t % 4, (t // 4) % 32
                        out_rows = psum_s[32 * g:32 * g + 32, :]
                        tp = (0, 32 * g)
                        srs = stypes[t]
                        for pi, el_t in enumerate(parts):
                            for ri, (ra, rb, typ) in enumerate(srs):
                                st = g not in strip_started
                                strip_started.add(g)
                                stop = (t == strip_last[g] and kc == 1
                                        and pi == len(parts) - 1
                                        and ri == len(srs) - 1)
                                nc.tensor.matmul(
                                    out_rows[:, ra:rb], w_sl(typ, kc, c),
                                    el_t[:, kc * TILE + ra:kc * TILE + rb],
                                    start=st, stop=stop,
                                    tile_position=tp, skip_group_check=True)

            for t in range(n_tiles):
                for fn in const_sched.get(t, ()):
                    fn()
                x2 = item_pool.tile([128, 2 * TILE], dt.bfloat16, tag="x")
                xoff = 0
                # early: Pool gets the even tiles (its queue is otherwise
                # empty; SP's first item rides behind mats q0)
                if t < 9:
                    eng = nc.gpsimd if t % 2 == 0 else nc.sync
                else:
                    eng = (nc.sync, nc.gpsimd, nc.scalar)[t % 3]
                eng.dma_start(
                    x2[:], items_d[:, t * 2 * TILE:(t + 1) * 2 * TILE])

                # both d-chunks side by side in one 2-bank PSUM tile.
                # Snake the (dc, kc) order across tiles: within a graph run
                # the weights repeat tile-to-tile, so the first matmul of
                # tile t reuses the weight the last matmul of t-1 loaded and
                # doesn't wait on a fresh LDWEIGHTS.
                y = ypool.tile([128, 2 * TILE], dt.float32, tag="y")
                n_p = len(pieces[t])
                combos = [(0, 0), (0, 1), (1, 1), (1, 0)]  # (dc, kc)
                if t % 2:
                    combos = combos[::-1]
                for pi, (a, b, u) in enumerate(pieces[t]):
                    for ci, (dc, kc) in enumerate(combos):
                        # ci 0/2 are each dc-half's first write: start=True
                        # marks that half's PSUM bank pending-zero
                        ysl = y[:, dc * TILE + a:dc * TILE + b]
                        nc.tensor.matmul(
                            ysl, mat_sl(u, kc, dc),
                            x2[:, xoff + kc * TILE + a:xoff + kc * TILE + b],
                            start=(pi == 0 and ci in (0, 2)),
                            stop=(pi == n_p - 1 and ci in (1, 3)),
                            skip_group_check=True)

                if not _is_b_tile(t, n_tiles):
                    # single-pass poly drain on DVE
                    el_t = elu_pool.tile([128, 2 * TILE], dt.bfloat16,
                                         tag="el")
                    nc.vector._custom_dve(poly, out=el_t[:], in0=y[:],
                                          s0=PC1, s1=PC2)
                    parts = [el_t]
                else:
                    # exact drain: elu(y)+1 = relu(y) + min(e^y, 1); both
                    # terms go straight to the s-reduce (it accumulates)
                    t1 = bt_pool.tile([128, 2 * TILE], dt.bfloat16, tag="t1")
                    nc.scalar.activation(t1[:], y[:],
                                         mybir.ActivationFunctionType.Relu)
                    e_t = bt_pool.tile([128, 2 * TILE], dt.bfloat16, tag="e")
                    nc.scalar.activation(e_t[:], y[:],
                                         mybir.ActivationFunctionType.Exp)
                    t2 = bt_pool.tile([128, 2 * TILE], dt.bfloat16, tag="t2")
                    nc.vector.tensor_scalar_min(t2[:], e_t[:], 1.0)
                    parts = [t1, t2]

                pending.append((t, parts))
                if len(pending) >= FLUSH_AT:
                    flush_batch()

            while pending:
                flush_batch()

            s_sb = sout_pool.tile([128, TILE], dt.float32)
            nc.vector.tensor_copy(out=s_sb[0:64, 0:1], in_=warm_ps[0:64, 0:1])
            # final drain on ACT: it goes idle a few us before the last
            # s-matmuls retire, while DVE still has poly drains queued
            nc.scalar.activation(s_sb[:], psum_s[:],
                                 mybir.ActivationFunctionType.Copy)
            nc.scalar.dma_start(s_d[:], s_sb[:])

    nc.compile()
    _BASS_CACHE[key] = nc
    return nc


# ----------------------------------------------------------------------------
# Host-side wrapper
# ----------------------------------------------------------------------------

def kernel(instruction_batch, distribution, node_prop_similarities,
           relation_similarity, node_attrs, edge_attrs,
           W_node_props, W_edge, w_node_score, w_rel_score,
           node_indices, edge_batch_indices, edge_indices):
    from concourse.bass_utils import run_bass_kernel_spmd

    ib = np.asarray(instruction_batch, dtype=np.float32)
    dist = np.asarray(distribution, dtype=np.float32)
    sim = np.asarray(node_prop_similarities, dtype=np.float32)
    rsim = np.asarray(relation_similarity, dtype=np.float32)
    na = np.asarray(node_attrs, dtype=np.float32)
    ea = np.asarray(edge_attrs, dtype=np.float32)
    Wp = np.asarray(W_node_props, dtype=np.float32)
    We = np.asarray(W_edge, dtype=np.float32)
    wn = np.asarray(w_node_score, dtype=np.float32)
    wr = np.asarray(w_rel_score, dtype=np.float32)
    ni = np.asarray(node_indices).astype(np.int64)
    ebi = np.asarray(edge_batch_indices).astype(np.int64)
    ei = np.asarray(edge_indices).astype(np.int64)
    src, dst = ei[0], ei[1]

    B = ib.shape[0]
    N = na.shape[0]
    G = B // N_CORES  # graphs (slots) per core

    cn = np.bincount(ni, minlength=B)
    ce = np.bincount(ebi, minlength=B)
    nstart = np.concatenate([[0], np.cumsum(cn)])
    eperm = np.argsort(ebi, kind="stable")
    estart = np.concatenate([[0], np.cumsum(ce)])

    # ---- layout plan: rank graphs by edge count, slot j = ranks [8j, 8j+8)
    order = np.argsort(-ce, kind="stable")
    slot_graphs = order.reshape(G, N_CORES)          # [slot, dev] -> graph
    Ln = (-(-cn[slot_graphs].max(axis=1) // 4)) * 4  # per-slot node run len
    Le = (-(-ce[slot_graphs].max(axis=1) // 4)) * 4

    # Region order [edges slots 8-15 | nodes 0-15 | edges 0-7]: the first
    # third references only mats chunk 0, so only 1 MB of weights is on the
    # startup critical path; the rest streams during the first regions.
    MINP = 4  # LDWEIGHTS pipelines under even tiny matmuls; no snap needed

    def place(lens, o0):
        offs, lens2 = [], []
        o = int(o0)
        for ln in lens:
            ln = int(ln)
            r = o % TILE
            if r and TILE - r < MINP:
                o += TILE - r
            end = o + ln
            tail = end % TILE
            if end // TILE > o // TILE and 0 < tail < MINP:
                ln += MINP - tail
            offs.append(o)
            lens2.append(ln)
            o += ln
        return offs, lens2, o

    eoff_hi, Le2_hi, r1 = place([Le[j] for j in range(G // 2, G)], 0)
    noff, Ln2, r2 = place(Ln, r1)
    eoff_lo, Le2_lo, total = place([Le[j] for j in range(G // 2)], r2)
    eoff = eoff_lo + eoff_hi                # [slot] -> column offset
    Le2 = Le2_lo + Le2_hi
    n_tiles = -(-total // TILE)
    m_pad = n_tiles * TILE
    assert n_tiles <= 128, "s accumulator bank overflow"

    def u_of(j, typ):                       # weight index in use order
        if typ == 0:
            return 8 + j
        return 24 + j if j < G // 2 else j - G // 2

    runs = [(noff[j], Ln2[j], u_of(j, 0)) for j in range(G)] + \
           [(eoff[j], Le2[j], u_of(j, 1)) for j in range(G)]
    pieces = [[] for _ in range(n_tiles)]
    for (st, ln, u) in runs:
        if ln == 0:
            continue
        for t in range(st // TILE, (st + ln - 1) // TILE + 1):
            a = max(st, TILE * t) - TILE * t
            b = min(st + ln, TILE * (t + 1)) - TILE * t
            pieces[t].append((a, b, u))
    for p in pieces:
        p.sort()
    stypes = []
    for t in range(n_tiles):
        sr = []
        for (lo, hi, typ) in ((0, r1, 1), (r1, r2, 0), (r2, m_pad, 1)):
            a = max(lo, TILE * t) - TILE * t
            b = min(hi, TILE * (t + 1)) - TILE * t
            if a < b:
                sr.append((a, b, typ))
        stypes.append(tuple(sr))
    pieces = tuple(tuple(p) for p in pieces)
    stypes = tuple(stypes)

    # ---- item columns, transposed + bf16, packed per plan ----
    na_bf = na.astype(BF16)
    ea_bf = ea[eperm].astype(BF16)
    itemsv = np.zeros((N_CORES, 128, n_tiles, 2, TILE), dtype=BF16)

    def put(dev, col0, block):
        n = block.shape[0]
        bT = block.T.reshape(2, 128, n)  # [kc, p, n]
        j = np.arange(col0, col0 + n)
        tt, jj = j // TILE, j % TILE
        itemsv[dev][:, tt, 0, jj] = bT[0]
        itemsv[dev][:, tt, 1, jj] = bT[1]

    for j in range(G):
        for d in range(N_CORES):
            g = int(slot_graphs[j, d])
            put(d, int(noff[j]), na_bf[nstart[g]:nstart[g + 1]])
            put(d, int(eoff[j]), ea_bf[estart[g]:estart[g + 1]])
    items = itemsv.reshape(N_CORES, 128, 2 * m_pad)

    # ---- per-graph matrices A[k, d] (instr folded in), bf16 ----
    C = np.einsum("gp,pde->gde", sim, Wp)
    A_node = (C * ib[:, :, None]).transpose(0, 2, 1)           # [g, k, d]
    A_edge = (We[None, :, :] * ib[:, :, None]).transpose(0, 2, 1)
    A_all = np.empty((N_CORES, 2 * G, D, D), np.float32)       # [dev, u, k, d]
    for j in range(G):
        for d in range(N_CORES):
            g = int(slot_graphs[j, d])
            A_all[d, u_of(j, 0)] = A_node[g]
            A_all[d, u_of(j, 1)] = A_edge[g]
    # blob[p, ((u*2+kc)*2+dc)*128 + m] = A_u[kc*128+p][dc*128+m]
    Ar = A_all.reshape(N_CORES, 2 * G, 2, 128, 2, 128)  # dev,u,kc,p,dc,m
    mats = np.ascontiguousarray(Ar.transpose(0, 3, 1, 2, 4, 5)
                                ).reshape(N_CORES, 128, -1).astype(BF16)

    # ---- w tables: wtab[k, ((c*2+typ)*2+kc)*32+m] = w_typ[kc*128+k]*(m==c)
    wt = np.stack([wn, wr]).astype(np.float32)                  # [2, 256]
    eye = np.eye(32, dtype=np.float32)
    wtab = np.einsum("tk,cm->kctm", wt.reshape(2, 2, 128).reshape(4, 128), eye)
    wtab = np.ascontiguousarray(wtab.reshape(128, 32, 2, 2, 32)
                                ).reshape(128, 4 * 32 * 32).astype(BF16)

    # ---- run on 8 cores ----
    nc = _build_bass(n_tiles, pieces, stypes)
    in_maps = [{"items": items[d], "mats": mats[d], "wtab": wtab}
               for d in range(N_CORES)]
    res = run_bass_kernel_spmd(nc, in_maps, core_ids=list(range(N_CORES)))
    s_rows = np.stack([r["s_out"] for r in res.results])        # [8, 128, 512]

    # ---- unshard + finish on host ----
    sum_wn = float(wt[0].astype(BF16).astype(np.float32).sum())
    sum_wr = float(wt[1].astype(BF16).astype(np.float32).sum())
    state_logits = np.empty(N, np.float32)
    s_e = np.empty(ei.shape[1], np.float32)
    # tile t lives at psum row 32*(t%4) + (t//4)%32
    row_of_tile = np.array([32 * (t % 4) + (t // 4) % 32
                            for t in range(n_tiles)])
    flat = s_rows.reshape(N_CORES, -1)

    def gather(dev, o, ln):
        j = np.arange(o, o + ln)
        return flat[dev][row_of_tile[j // TILE] * TILE + (j % TILE)]

    for j in range(G):
        for d in range(N_CORES):
            g = int(slot_graphs[j, d])
            state_logits[nstart[g]:nstart[g + 1]] = \
                gather(d, int(noff[j]), int(cn[g])) - sum_wn
            s_e[estart[g]:estart[g + 1]] = \
                gather(d, int(eoff[j]), int(ce[g])) - sum_wr

    rel_logits = np.bincount(dst[eperm], weights=dist[src[eperm]] * s_e,
                             minlength=N).astype(np.float32)

    def seg_softmax(x):
        mx = np.maximum.reduceat(x, nstart[:-1])
        ex = np.exp(x - mx[ni])
        sm = np.add.reduceat(ex, nstart[:-1])
        return ex / sm[ni]

    r = rsim[ni]
    out = r * seg_softmax(rel_logits) + (1.0 - r) * seg_softmax(state_logits)
    return out.astype(np.float32)
